# revision 37
# baseline (speedup 1.0000x reference)
"""NodeConv kernel for 8 Trainium2 NeuronCores.

Reference computes, for adj [B,1,N,N], node [B,nin,N], Wi/Wj [nout,nin]:
    x  = node[:, :, None, :] * adj          # [B,nin,N,N]
    yi = einsum('oc,bcij->boij', Wi, x)
    yj = einsum('oc,bcij->boij', Wj, x)
    out = I * yi + (1-I) * yj

Because adj[b,i,j] does not depend on the contraction channel c, the
contraction factors out:
    off-diag: out[b,o,i,j] = adj[b,i,j] * (Wj @ node[b])[o,j]
    diag:     out[b,o,j,j] = adj[b,j,j] * (Wi @ node[b])[o,j]

So per batch we need two tiny matmuls (u = Wj@node, v = Wi@node) and a
broadcast multiply out[o,i,j] = adj[i,j]*u[o,j] with a diagonal patch.
The output write is the memory roofline; the device stores a bf16 plane
(half the HBM traffic) and the host upcasts to f32 — bf16 rounding plus
one bf16 adj term plus f32r head matmuls leave ~5e-3 relative error,
well inside the 2e-2 gate.

Sharding: core c handles batch b=c//2, row half h=c%2 (128 rows). Odd
halves get their columns rolled by -128 on the host so the diagonal of
local row l sits at local column l on every core -> one SPMD program;
the host rolls the output back while gathering.

Per-core device program (measured ~56.3us on HW):
  - u = Wj @ node_r, v = Wi @ node_r[:, :128], dv = adj_diag * v; these
    head matmuls run in f32r (tf32-like, 1 cycle/row vs fp32's 4).
  - per 8-row chunk p (16 chunks): PE broadcasts the 8 adj rows (bf16,
    NTERMS terms in the contraction dim) to all 128 partitions via
    one-hot-selector matmuls into PSUM; DVE multiplies by u replicated
    8x (stride-0 view) writing the bf16 output tile (2.2us/chunk — DVE
    is the production pole); ScalarE patches the 8 diagonal elements
    via a stride-257 view.
  - stores: 4 groups [6,5,3,2] alternating the two HWDGE queues.  The
    HWDGE descriptor processor rides DMA engine 15 and throttles when
    it handles too many descriptors (>~700 total at 16 descs/us makes
    e15 ~20% slower per byte and its backlog spills serially past the
    end), so groups are few and large: 512 output descriptors total,
    with a small tail group so the last multiply's store drains fast.
"""

import os

import numpy as np

NCORES = 8
B, N, NIN, NOUT = 4, 256, 128, 128
RPC = 128          # rows per core
CH = 16            # chunks per core
RCH = 8            # rows per chunk
FREE = RCH * N     # 2048 free elems per chunk

NTERMS = int(os.environ.get("NODECONV_NTERMS", "1"))   # bf16 terms (2 or 3)
OUT_BUFS = int(os.environ.get("NODECONV_OUT_BUFS", "3"))
# Store group sizes.  Ramped: a small head group starts the DMA stream as
# early as possible, large middle groups cut descriptor count (the HWDGE
# ring processor on DMA engine 15 fetches descriptors at ~44ns each and
# throttles total bandwidth when descriptors are small), and a small tail
# group minimizes the unoverlapped drain after the last chunk's multiply.
_G = [
    int(x)
    for x in os.environ.get("NODECONV_G", "6,5,3,2").split(",")
]
assert sum(_G) == CH
GMAX = max(_G)
_STORE_ENG = os.environ.get("NODECONV_STORE_ENG", "alt")
# Stage PSUM->SBUF on ScalarE so the DVE multiply reads all-SBUF operands
# (DVE 2-port mode: ~2x elem rate, no dtype requirement); diag patch moves
# to DVE to keep ScalarE below the production pole.
_STAGE = os.environ.get("NODECONV_STAGE", "0") == "1"
_LAST_DIAG_DVE = os.environ.get("NODECONV_LAST_DIAG_DVE", "0") == "1"
# chunks whose multiply runs on GpSimd (staged through ScalarE PSUM->SBUF
# copy).  Measured WORSE than all-DVE (SBUF port interference: 64 -> 72us on
# affected cores), so default off; kept as a knob for experiments.
_GP_CHUNKS = {
    int(x)
    for x in os.environ.get("NODECONV_GP_CHUNKS", "").split(",")
    if x != ""
}

KP = CH * NTERMS   # contraction partitions of the broadcast matmul

_cached = {}

last_results = None  # BassKernelResults of the most recent kernel() call


def _build_nc():
    key = (NTERMS, tuple(_G), OUT_BUFS, _STORE_ENG, _STAGE, _LAST_DIAG_DVE)
    if key in _cached:
        return _cached[key]

    from contextlib import ExitStack

    import concourse.tile as tile
    from concourse import bacc, mybir

    f32 = mybir.dt.float32
    f32r = mybir.dt.float32r
    bf16 = mybir.dt.bfloat16

    nc = bacc.Bacc(
        "TRN2", target_bir_lowering=False, debug=False, num_devices=NCORES
    )

    # pk: [KP, 2*FREE] bf16 — adj terms in [:, :FREE], one-hot selector
    # blocks in [:, FREE:]
    pk = nc.dram_tensor("pk", [KP, 2 * FREE], bf16, kind="ExternalInput").ap()
    # ckf: [128, 512] f32 — node_r | WiT | WjT
    ckf = nc.dram_tensor("ckf", [NIN, N + 2 * NOUT], f32r, kind="ExternalInput").ap()
    # dsz: [16, 256] f32 — diag row | f32 one-hot (selects partition 0)
    dsz = nc.dram_tensor("dsz", [CH, 2 * RPC], f32r, kind="ExternalInput").ap()
    # bf16 output plane: halves the HBM store traffic (the memory roofline);
    # the host upcasts to f32.  bf16 rounding adds <=2^-9 relative error,
    # well inside the 2e-2 gate.
    out = nc.dram_tensor("out", [NOUT, RPC * N], bf16, kind="ExternalOutput").ap()

    with tile.TileContext(nc) as tc, ExitStack() as ctx:
        const = ctx.enter_context(tc.tile_pool(name="const", bufs=1))
        psum = ctx.enter_context(tc.tile_pool(name="psum", bufs=2, space="PSUM"))
        outp = ctx.enter_context(tc.tile_pool(name="outp", bufs=OUT_BUFS))
        stage = (
            ctx.enter_context(tc.tile_pool(name="stage", bufs=2))
            if (_GP_CHUNKS or _STAGE)
            else None
        )

        # pk first, then ckf, both on the sync queue: pk gates the chunk-0
        # broadcast matmuls (the longer pole to TT0); ckf's u chain overlaps
        # pk's transfer.  dsz rides the scalar queue.
        _ord = os.environ.get("NODECONV_IN_ORDER", "pk_sync")
        pk_sb = const.tile([KP, 2 * FREE], bf16)
        ckf_sb = const.tile([NIN, N + 2 * NOUT], f32r)
        dsz_sb = const.tile([CH, 2 * RPC], f32r)
        if _ord == "pk_gp":
            # pk via SWDGE: its completion signal takes the software-DGE
            # path, dodging the ~2.5us HWDGE completion-sem latency
            nc.gpsimd.dma_start(out=pk_sb[:], in_=pk)
            nc.sync.dma_start(out=ckf_sb[:], in_=ckf)
            nc.scalar.dma_start(out=dsz_sb[:], in_=dsz)
        elif _ord == "pk_sync":
            # pk's completion sem gates TT0 (the chunk-0 broadcast); the
            # sync queue delivers it ~1.4us sooner than scalar
            nc.sync.dma_start(out=pk_sb[:], in_=pk)
            nc.scalar.dma_start(out=ckf_sb[:], in_=ckf)
            nc.scalar.dma_start(out=dsz_sb[:], in_=dsz)
        elif _ord == "pk_first":
            nc.sync.dma_start(out=pk_sb[:], in_=pk)
            nc.sync.dma_start(out=ckf_sb[:], in_=ckf)
            nc.scalar.dma_start(out=dsz_sb[:], in_=dsz)
        else:
            nc.sync.dma_start(out=ckf_sb[:], in_=ckf)
            nc.scalar.dma_start(out=pk_sb[:], in_=pk)
            nc.scalar.dma_start(out=dsz_sb[:], in_=dsz)

        node_sb = ckf_sb[:, 0:N]
        wit_sb = ckf_sb[:, N : N + NOUT]
        wjt_sb = ckf_sb[:, N + NOUT : N + 2 * NOUT]
        diag_sb = dsz_sb[:, 0:RPC]
        selz_sb = dsz_sb[:, RPC : 2 * RPC]

        # u = Wj @ node_r  -> [nout, N].  f32r (tf32-like) runs the PE at
        # 1 cycle/row instead of fp32's 4 passes; ~1e-3 relative precision
        # is far inside the 2e-2 gate.
        ps_u = psum.tile([NOUT, N], f32, tag="mm")
        nc.tensor.matmul(ps_u[:], lhsT=wjt_sb, rhs=node_sb, start=True, stop=True)
        u_sb = const.tile([NOUT, N], f32)
        nc.scalar.copy(u_sb[:], ps_u[:])

        # v = Wi @ node_r[:, :128]  (only the diagonal columns are needed)
        ps_v = psum.tile([NOUT, RPC], f32, tag="mm")
        nc.tensor.matmul(
            ps_v[:], lhsT=wit_sb, rhs=node_sb[:, 0:RPC], start=True, stop=True
        )
        v_sb = const.tile([NOUT, RPC], f32)
        nc.scalar.copy(v_sb[:], ps_v[:])

        # dv[o,l] = adj_diag[l] * v[o,l]; broadcast diag row over partitions
        ps_d = psum.tile([NOUT, RPC], f32, tag="mm")
        nc.tensor.matmul(ps_d[:], lhsT=selz_sb, rhs=diag_sb, start=True, stop=True)
        dv_sb = const.tile([NOUT, RPC], f32)
        nc.vector.tensor_mul(dv_sb[:], ps_d[:], v_sb[:])

        # u replicated RCH times along the free dim via a stride-0 view
        u_rep = u_sb[:].unsqueeze(1).broadcast_to([NOUT, RCH, N])
        if _STAGE or _LAST_DIAG_DVE:
            z8_sb = const.tile([NOUT, RCH], f32)
            nc.gpsimd.memset(z8_sb[:], 0.0)
        if _GP_CHUNKS:
            # bf16 copy of u for the GpSimd multiply path
            ub_sb = const.tile([NOUT, N], bf16)
            nc.scalar.copy(ub_sb[:], u_sb[:])
            ub_rep = ub_sb[:].unsqueeze(1).broadcast_to([NOUT, RCH, N])

        H = FREE // 2
        split_c0 = os.environ.get("NODECONV_SPLIT_C0", "0") == "1"
        p = 0
        for gi, gsz in enumerate(_G):
            # fixed-size shell so the pool stays rectangular across groups
            o_sb = outp.tile([NOUT, GMAX * FREE], bf16, tag="osb")
            p0 = p
            for g in range(gsz):
                ps_b = psum.tile([NOUT, FREE], f32, tag="mm")
                lhs = pk_sb[:, FREE + NOUT * p : FREE + NOUT * (p + 1)]
                if p == 0 and split_c0:
                    # chunk 0 split in two halves so the first DVE multiply
                    # starts after 2 broadcast matmuls instead of 4 — pulls
                    # the whole production conveyor ~1us earlier
                    u_rep_h = u_sb[:].unsqueeze(1).broadcast_to(
                        [NOUT, RCH // 2, N]
                    )
                    for h in range(2):
                        for q in range(2 * h, 2 * h + 2):
                            sl = slice(512 * q, 512 * (q + 1))
                            nc.tensor.matmul(
                                ps_b[:, sl],
                                lhsT=lhs,
                                rhs=pk_sb[:, sl],
                                start=True,
                                stop=True,
                            )
                        nc.vector.tensor_mul(
                            o_sb[:, h * H : (h + 1) * H].rearrange(
                                "p (k j) -> p k j", k=RCH // 2
                            ),
                            ps_b[:, h * H : (h + 1) * H].rearrange(
                                "p (k j) -> p k j", k=RCH // 2
                            ),
                            u_rep_h,
                        )
                        # diag of local row k (k = 4h..4h+3) sits at free
                        # offset k*257; half h starts at 4h*257 = h*(H+4)
                        d0 = h * (H + RCH // 2)
                        nc.scalar.copy(
                            o_sb[
                                :,
                                d0 : d0 + (RCH // 2 - 1) * (N + 1) + 1 : N + 1,
                            ],
                            dv_sb[:, RCH // 2 * h : RCH // 2 * (h + 1)],
                        )
                    p += 1
                    continue
                for q in range(FREE // 512):
                    sl = slice(512 * q, 512 * (q + 1))
                    nc.tensor.matmul(
                        ps_b[:, sl], lhsT=lhs, rhs=pk_sb[:, sl], start=True, stop=True
                    )
                o_view = o_sb[:, g * FREE : (g + 1) * FREE].rearrange(
                    "p (k j) -> p k j", k=RCH
                )
                diag_view = o_sb[
                    :,
                    g * FREE + RCH * p : g * FREE
                    + RCH * p
                    + (RCH - 1) * (N + 1)
                    + 1 : N + 1,
                ]
                if _STAGE:
                    # all-SBUF multiply: ScalarE stages PSUM->SBUF f32 so the
                    # DVE TT reads via both SBUF ports (2x elem rate); diag
                    # patch rides DVE to keep ScalarE under the pole
                    st_sb = stage.tile([NOUT, FREE], f32, tag="st")
                    nc.scalar.copy(st_sb[:], ps_b[:])
                    nc.vector.tensor_mul(
                        o_view, st_sb[:].rearrange("p (k j) -> p k j", k=RCH), u_rep
                    )
                    nc.vector.tensor_add(
                        diag_view, dv_sb[:, RCH * p : RCH * (p + 1)], z8_sb[:]
                    )
                elif p in _GP_CHUNKS:
                    # all-bf16 staged path: ScalarE casts PSUM->SBUF bf16,
                    # GpSimd multiplies bf16*bf16->bf16
                    st_sb = stage.tile([NOUT, FREE], bf16, tag="st")
                    nc.scalar.copy(st_sb[:], ps_b[:])
                    nc.gpsimd.tensor_mul(
                        o_view, st_sb[:].rearrange("p (k j) -> p k j", k=RCH), ub_rep
                    )
                    nc.scalar.copy(diag_view, dv_sb[:, RCH * p : RCH * (p + 1)])
                    p += 1
                    continue
                else:
                    nc.vector.tensor_mul(
                        o_view, ps_b[:].rearrange("p (k j) -> p k j", k=RCH), u_rep
                    )
                if not _STAGE:
                    # diagonal of local row l=8p+k sits at free offset 8p+k*257
                    if p == CH - 1 and _LAST_DIAG_DVE:
                        # last chunk: patch on DVE right after its own TT —
                        # drops the ScalarE round-trip from the final store's
                        # critical path
                        nc.vector.tensor_add(
                            diag_view,
                            dv_sb[:, RCH * p : RCH * (p + 1)],
                            z8_sb[:],
                        )
                    else:
                        nc.scalar.copy(
                            diag_view, dv_sb[:, RCH * p : RCH * (p + 1)]
                        )
                p += 1
            if _STORE_ENG == "gp":
                # SWDGE: descriptor ring lives in the SBUF carveout, so
                # descriptor fetches bypass the HBM-congested path that
                # throttles the HWDGE ring processor (DMA engine 15).
                eng = nc.gpsimd
            else:
                eng = nc.sync if gi % 2 == 0 else nc.scalar
            eng.dma_start(
                out=out[:, FREE * p0 : FREE * p], in_=o_sb[:, 0 : gsz * FREE]
            )

    nc.compile()
    _cached[key] = nc
    return nc


def _split_terms(x, nterms):
    """Split fp32 array into bf16 terms whose fp32 sum approximates x.
    2 terms leave <=2^-18 relative error; 3 terms are exact."""
    import ml_dtypes

    terms = []
    r = x
    for _ in range(nterms):
        t = r.astype(ml_dtypes.bfloat16)
        terms.append(t)
        r = (r - t.astype(np.float32)).astype(np.float32)
    return terms


def _in_maps(adj, node, Wi, Wj):
    import ml_dtypes

    bf16 = ml_dtypes.bfloat16
    sel = np.zeros((KP, CH * NOUT), bf16)
    for p in range(CH):
        for t in range(NTERMS):
            sel[CH * t + p, NOUT * p : NOUT * (p + 1)] = 1.0
    dszz = np.zeros((CH, 2 * RPC), np.float32)
    dszz[0, RPC : 2 * RPC] = 1.0
    ckf = np.empty((NIN, N + 2 * NOUT), np.float32)
    ckf[:, N : N + NOUT] = Wi.T
    ckf[:, N + NOUT :] = Wj.T
    maps = []
    for c in range(NCORES):
        b, h = divmod(c, 2)
        r0 = RPC * h
        a = adj[b, 0, r0 : r0 + RPC, :]
        dsz = dszz.copy()
        dsz[0, 0:RPC] = a[np.arange(RPC), r0 + np.arange(RPC)]
        if h:
            ar = np.roll(a, -r0, axis=1)
            noder = np.roll(node[b], -r0, axis=1)
        else:
            ar = a
            noder = node[b]
        pk = np.empty((KP, 2 * FREE), bf16)
        terms = _split_terms(ar.reshape(CH, FREE), NTERMS)
        for t in range(NTERMS):
            pk[CH * t : CH * (t + 1), 0:FREE] = terms[t]
        pk[:, FREE:] = sel
        m_ckf = ckf.copy()
        m_ckf[:, 0:N] = noder
        maps.append({"pk": pk, "ckf": m_ckf, "dsz": dsz})
    return maps


def kernel(**inputs):
    global last_results
    adj = np.asarray(inputs["adj"], dtype=np.float32)
    node = np.asarray(inputs["node"], dtype=np.float32)
    Wi = np.asarray(inputs["Wi"], dtype=np.float32)
    Wj = np.asarray(inputs["Wj"], dtype=np.float32)

    from concourse.bass_utils import run_bass_kernel_spmd

    nc = _build_nc()
    res = run_bass_kernel_spmd(nc, _in_maps(adj, node, Wi, Wj), list(range(NCORES)))
    last_results = res

    out = np.empty((B, NOUT, N, N), np.float32)
    for c in range(NCORES):
        b, h = divmod(c, 2)
        # device writes a bf16 plane; upcast to f32 on the host
        co = res.results[c]["out"].astype(np.float32).reshape(NOUT, RPC, N)
        if h:
            co = np.roll(co, RPC * h, axis=2)
        out[b, :, RPC * h : RPC * (h + 1), :] = co
    return out

